# revision 58
# baseline (speedup 1.0000x reference)
"""ArbSR (moe_routing) Trainium2 kernel, 8-core SPMD.

Structure exploited: with scale=4, the scale-embedding MLP input is periodic
with period 4 in both HR axes, so routing r, offsets off, and the expert-mix
matrices take only 16 distinct values (one per (y%4, x%4) class).  The
offset grid_sample is, per class, a 2x2-tap bilinear filter of the encoder
feature map at a constant integer shift, so the whole
  encoder conv -> fea0 -> expert mixing -> (+fea0) -> 3x3 tail conv
chain is LINEAR in the input image and collapses to a single 5x5-tap
convolution per (y%4, x%4) class:
  pred[:, 4*yl+b, 4*xl+a] = bias + sum_{ey,ex in 5x5} A[(ey,ex)] @
                            inp[:, yl+ey-2, xl+ex-2]
with host-precomputed [48, 3] matrices A (deltas composed with encoder
taps).  The device runs ONE K=84 matmul per bank of 4 LR rows (4 mains
total, N=512 each) against a 25-tap im2col of the raw input; the bias rides
an all-ones rhs K row.  The fold is exact at interior pixels; at image
borders (where the grid-sample zero-pad mask breaks linear folding) the
host computes exact (true - fold) corrections numerically: left/right
columns ride one-hot rhs K rows, top/bottom rows (cores 0/7) are
accumulated into PSUM by an identity-lhsT matmul against a [48, W] strip.

Per core (64 HR rows): 4 main + 2 edge matmuls, per-bank PSUM drains split
across DVE/ACT, and 4 direct class-major output writes; the host resolves
the nearest-neighbour queries from the dumped pred (query index math is
host-side, as in the original baseline).
"""

import numpy as np
import ml_dtypes

BF16 = ml_dtypes.bfloat16


def _ensure_path():
    import sys
    for p in ('/opt/trn_rl_repo',):
        if p not in sys.path:
            sys.path.append(p)


H = W = 128
S = 4
HH = WH = H * S          # 512
C = 64
NCORES = 8
YLC = H // NCORES        # 16 LR rows per core
HRPC = HH // NCORES      # 64 HR rows per core
NPIX = HRPC * WH         # 32768 HR pixels per core
NCLS = 16                # (b, a) classes
MROWS = NCLS * 3         # 48 stacked pred rows
KE = 28                  # encoder-folded contraction rows per dy block
NROWS_B = 2048           # D rows per bank (128 xl * 4 t * 4 g)

GATHER = False           # False: dump D as output, gather on host
EDGE_IN_MAIN = True      # fold top/bot row corr into the main PSUM group
T_SHARED = True          # transposes share one PSUM tile per bank
SPLIT_DRAIN = True       # split PSUM drains between Vector and Scalar
NO_SCALAR = False        # no scalar.activation at all: kills the 1.3us
                         # ACT_TABLE_LOAD that blocks scalar's DMA queue
WARMUP_MM = 3            # garbage matmuls while input DMAs land, to
                         # trigger the PE HAM un-throttle (1.2->2.4 GHz)
IMKA_SYNC = False        # imk chunk A on sync instead of gpsimd


def _sigmoid(x):
    return 1.0 / (1.0 + np.exp(-x))


def _class_constants(d):
    w1 = np.asarray(d['body_w1'], np.float64)
    b1 = np.asarray(d['body_b1'], np.float64)
    w2 = np.asarray(d['body_w2'], np.float64)
    b2 = np.asarray(d['body_b2'], np.float64)
    rw = np.asarray(d['routing_w'], np.float64)
    rb = np.asarray(d['routing_b'], np.float64)
    ow = np.asarray(d['offset_w'], np.float64)
    ob = np.asarray(d['offset_b'], np.float64)
    wc = np.asarray(d['weight_compress'], np.float64)
    we = np.asarray(d['weight_expand'], np.float64)

    fs = float(S)
    coor = np.array([(i + 0.5) / fs - np.floor((i + 0.5) / fs + 0.001) - 0.5
                     for i in range(S)])
    cls = {}
    for b in range(S):
        for a in range(S):
            inp4 = np.array([1.0 / fs, 1.0 / fs, coor[b], coor[a]])
            emb = np.maximum(w1 @ inp4 + b1, 0.0)
            emb = np.maximum(w2 @ emb + b2, 0.0)
            off = ow @ emb + ob
            r = _sigmoid(rw @ emb + rb)
            A = np.einsum('e,eck->ck', r, we) @ np.einsum('e,ekc->kc', r, wc)
            B = A + np.eye(C)
            cx = (a + 0.5) / fs - 0.5 + off[0]
            cy = (b + 0.5) / fs - 0.5 + off[1]
            ix, iy = int(np.floor(cx)), int(np.floor(cy))
            fx, fy = cx - ix, cy - iy
            wbl = {(0, 0): (1 - fy) * (1 - fx), (0, 1): (1 - fy) * fx,
                   (1, 0): fy * (1 - fx), (1, 1): fy * fx}
            cls[(b, a)] = dict(B=B, ix=ix, iy=iy, wbl=wbl)
    return cls


def _build_E(tail_w, cls):
    """E[(b,a)][(dy,dx)] = [3, C] so that pred contribution is E @ f(shift)."""
    Es = {}
    for b in range(S):
        for a in range(S):
            acc = {}
            for ty in range(3):
                for tx in range(3):
                    bp = (b + ty - 1) % S
                    oy = (b + ty - 1 - bp) // S
                    ap_ = (a + tx - 1) % S
                    ox = (a + tx - 1 - ap_) // S
                    c2 = cls[(bp, ap_)]
                    TB = tail_w[:, :, ty, tx] @ c2['B']
                    for (uy, ux), wgt in c2['wbl'].items():
                        if wgt == 0.0:
                            continue
                        key = (oy + c2['iy'] + uy, ox + c2['ix'] + ux)
                        acc[key] = acc.get(key, np.zeros((3, C))) + TB * wgt
            Es[(b, a)] = acc
    return Es


def _build_encw(d):
    """encw [28, C]: rows = 9 taps x 3 ch + inside-mask bias row."""
    enc_w = np.asarray(d['enc_w'], np.float64)
    enc_b = np.asarray(d['enc_b'], np.float64)
    encw = np.zeros((KE, C))
    for ty in range(3):
        for tx in range(3):
            for ch in range(3):
                encw[(ty * 3 + tx) * 3 + ch, :] = enc_w[:, ch, ty, tx]
    encw[27, :] = enc_b
    return encw


PADF = 4   # f64 f-map padding margin (covers all shift indexing)


def _host_f(d):
    """f64 encoder output, zero outside the image, with PADF margin."""
    inp = np.asarray(d['inp'], np.float64)[0]
    ip = np.pad(inp, ((0, 0), (1, 1), (1, 1)))
    enc_w = np.asarray(d['enc_w'], np.float64)
    enc_b = np.asarray(d['enc_b'], np.float64)
    f = np.zeros((C, H, W))
    for ty in range(3):
        for tx in range(3):
            f += np.einsum('oc,chw->ohw', enc_w[:, :, ty, tx],
                           ip[:, ty:ty + H, tx:tx + W])
    f += enc_b[:, None, None]
    fpad = np.zeros((C, H + 2 * PADF, W + 2 * PADF))
    fpad[:, PADF:PADF + H, PADF:PADF + W] = f
    return fpad


def _host_f_nomask(d):
    """f64 encoder output WITHOUT the zero-outside mask: enc taps on the
    zero-padded input + enc_b everywhere.  This is what the 5x5 fold
    computes; with PADF margin."""
    inp = np.asarray(d['inp'], np.float64)[0]
    P1 = PADF + 1
    ip = np.pad(inp, ((0, 0), (P1, P1), (P1, P1)))
    enc_w = np.asarray(d['enc_w'], np.float64)
    enc_b = np.asarray(d['enc_b'], np.float64)
    NE = H + 2 * PADF
    f = np.zeros((C, NE, NE))
    for ty in range(3):
        for tx in range(3):
            f += np.einsum('oc,chw->ohw', enc_w[:, :, ty, tx],
                           ip[:, ty:ty + NE, tx:tx + NE])
    f += enc_b[:, None, None]
    return f


def _zgrid(cls, fpad, yHs, xHs):
    """z = out2 + fea0 (zero outside the HR image) on a coordinate grid."""
    yh = np.asarray(yHs)
    xh = np.asarray(xHs)
    out = np.zeros((C, len(yh), len(xh)))
    for b_ in range(S):
        rm = np.nonzero(np.mod(yh, S) == b_)[0]
        if rm.size == 0:
            continue
        ys = yh[rm]
        yl = ys // S
        for a_ in range(S):
            cm = np.nonzero(np.mod(xh, S) == a_)[0]
            if cm.size == 0:
                continue
            xs = xh[cm]
            xl = xs // S
            inside = ((ys[:, None] >= 0) & (ys[:, None] < HH)
                      & (xs[None, :] >= 0) & (xs[None, :] < WH))
            c2 = cls[(b_, a_)]
            fg = np.zeros((C, rm.size, cm.size))
            for (uy, ux), wgt in c2['wbl'].items():
                rr = PADF + yl + c2['iy'] + uy
                cc = PADF + xl + c2['ix'] + ux
                fg += wgt * fpad[:, rr[:, None], cc[None, :]]
            val = np.einsum('oc,cyx->oyx', c2['B'], fg)
            val *= inside[None]
            out[np.ix_(np.arange(C), rm, cm)] = val
    return out


def _true_strip(cls, fpad, tail_w, tail_b, ylgs, xs_lr):
    """Exact pred values [MROWS, len(ylgs), len(xs_lr)] (LR coords)."""
    ylgs = np.asarray(ylgs)
    xs_lr = np.asarray(xs_lr)
    yHs = np.arange(S * ylgs.min() - 1, S * ylgs.max() + S + 1)
    xHs = np.arange(S * xs_lr.min() - 1, S * xs_lr.max() + S + 1)
    z = _zgrid(cls, fpad, yHs, xHs)
    y0, x0 = yHs[0], xHs[0]
    out = np.zeros((MROWS, len(ylgs), len(xs_lr)))
    for b in range(S):
        for a in range(S):
            m0 = (4 * b + a) * 3
            acc = np.zeros((3, len(ylgs), len(xs_lr)))
            for ty in range(3):
                rr = S * ylgs + b + ty - 1 - y0
                for tx in range(3):
                    cc = S * xs_lr + a + tx - 1 - x0
                    acc += np.einsum(
                        'oc,cyx->oyx', tail_w[:, :, ty, tx],
                        z[:, rr[:, None], cc[None, :]])
            out[m0:m0 + 3] = acc + tail_b[:, None, None]
    return out


def _fold_strip(G_main, bias48, fpad, ylgs, xs_lr):
    """What the device mains+bias compute, in f64 (LR coords)."""
    ylgs = np.asarray(ylgs)
    xs_lr = np.asarray(xs_lr)
    out = np.zeros((MROWS, len(ylgs), len(xs_lr)))
    for (dy, dx), G in G_main.items():
        rr = PADF + ylgs + dy
        cc = PADF + xs_lr + dx
        out += np.einsum('mc,cyx->myx', G, fpad[:, rr[:, None], cc[None, :]])
    return out + bias48[:, None, None]


def _plan_and_host_data(d):
    """Host precompute: folded lhsT matrices, per-core im2colK, edge
    corrections, query routing."""
    cls = _class_constants(d)
    tail_w = np.asarray(d['tail_w'], np.float64)
    tail_b = np.asarray(d['tail_b'], np.float64)
    encw = _build_encw(d)

    E_main = _build_E(tail_w, cls)
    deltas = sorted({k for acc in E_main.values() for k in acc})
    dys = sorted({dl[0] for dl in deltas})
    dxs = sorted({dl[1] for dl in deltas})
    dy_min, dy_max = min(dys), max(dys)
    dx_min, dx_max = min(dxs), max(dxs)
    assert dy_min >= -1 and dy_max <= 1 and dx_min >= -1 and dx_max <= 1
    # full 5x5 fold: delta (3x3) composed with the encoder taps (3x3)
    # gives pred = sum_{ey,ex in 5x5} A[ey,ex] @ inp + bias, valid at
    # interior pixels (the f zero-pad mask is 1); borders are fixed by
    # the numeric true-fold corrections
    KA = 75                          # 25 input taps x 3 channels
    KM = KA + 1 + 8                  # + bias row + lef/rig one-hots
    NCF = W                          # no column margins: taps carry them
    NFK = YLC * NCF

    G_main = {dl: np.zeros((MROWS, C)) for dl in deltas}
    for (b, a), acc in E_main.items():
        m0 = (4 * b + a) * 3
        for dl, M in acc.items():
            G_main[dl][m0:m0 + 3, :] += M

    bias48 = np.zeros(MROWS)
    for b in range(S):
        for a in range(S):
            bias48[(4 * b + a) * 3:(4 * b + a) * 3 + 3] = tail_b

    # ---- exact edge corrections (true - fold), f64 on host ----
    fpad = _host_f(d)
    fnom = _host_f_nomask(d)
    all_yl = np.arange(H)
    t_lef = _true_strip(cls, fpad, tail_w, tail_b, all_yl, [0])[:, :, 0]
    t_rig = _true_strip(cls, fpad, tail_w, tail_b, all_yl, [W - 1])[:, :, 0]
    f_lef = _fold_strip(G_main, bias48, fnom, all_yl, [0])[:, :, 0]
    f_rig = _fold_strip(G_main, bias48, fnom, all_yl, [W - 1])[:, :, 0]
    corrL = t_lef - f_lef            # [MROWS, H]
    corrR = t_rig - f_rig
    all_x = np.arange(W)
    t_top = _true_strip(cls, fpad, tail_w, tail_b, [0], all_x)[:, 0, :]
    f_top = _fold_strip(G_main, bias48, fnom, [0], all_x)[:, 0, :]
    corrT = t_top - f_top            # [MROWS, W]
    corrT[:, 0] -= corrL[:, 0]
    corrT[:, W - 1] -= corrR[:, 0]
    t_bot = _true_strip(cls, fpad, tail_w, tail_b, [H - 1], all_x)[:, 0, :]
    f_bot = _fold_strip(G_main, bias48, fnom, [H - 1], all_x)[:, 0, :]
    corrB = t_bot - f_bot
    corrB[:, 0] -= corrL[:, H - 1]
    corrB[:, W - 1] -= corrR[:, H - 1]

    # ---- main lhsT: one [KM, MROWS] via 5x5 A-matrices ----
    # row (ey*5+ex)*3+c accumulates F[(ty*3+tx)*3+c] over all
    # (dy,dx),(ty,tx) with ey=dy+ty-1+2, ex=dx+tx-1+2; F row 27
    # (enc-bias term, mask==1 interior) folds into the bias row
    main_lhsT = np.zeros((KM, MROWS))
    bias_row = bias48.copy()
    for (dy, dx), G in G_main.items():
        F = encw @ G.T
        for ty in range(3):
            for tx in range(3):
                ey, ex = dy + ty - 1 + 2, dx + tx - 1 + 2
                for c in range(3):
                    main_lhsT[(ey * 5 + ex) * 3 + c, :] += \
                        F[(ty * 3 + tx) * 3 + c, :]
        bias_row += F[27, :]
    main_lhsT[KA, :] = bias_row

    # ---- per-core im2col5 [KM, YLC*W]: 25 raw-input taps ----
    inp = np.asarray(d['inp'], np.float64)[0]   # [3, H, W]
    PADX = 8
    ippad = np.pad(inp, ((0, 0), (PADX, PADX), (PADX, PADX)))
    im2cols = []
    for core in range(NCORES):
        y0 = YLC * core
        imk = np.zeros((KM, YLC, NCF), np.float32)
        for ey in range(5):
            for ex in range(5):
                ys = PADX + y0 + ey - 2
                xs = PADX + ex - 2
                for ch in range(3):
                    imk[(ey * 5 + ex) * 3 + ch] = \
                        ippad[ch, ys:ys + YLC, xs:xs + NCF]
        imk[KA] = 1.0                              # bias row
        for yl in range(YLC):                      # lef/rig one-hot rows
            imk[KA + 1 + yl % 4, yl, 0] = 1.0
            imk[KA + 5 + yl % 4, yl, W - 1] = 1.0
        im2cols.append(imk.reshape(KM, NFK).astype(BF16))

    # ---- query routing (f32 math matches reference rounding) ----
    coord = np.asarray(d['coord'], np.float32)[0]
    cell = np.asarray(d['cell'], np.float32)[0]
    cq = np.clip(coord - cell * np.float32(0.5) + np.float32(1e-6),
                 np.float32(-1 + 1e-6), np.float32(1 - 1e-6))
    xi = np.clip(np.round((cq[:, 1] + 1) * np.float32(0.5) * (WH - 1)
                          ).astype(np.int64), 0, WH - 1)
    yi = np.clip(np.round((cq[:, 0] + 1) * np.float32(0.5) * (HH - 1)
                          ).astype(np.int64), 0, HH - 1)
    core_of = yi // HRPC
    ylq = (yi % HRPC) // S
    bq = yi % S
    xlq = xi // S
    aq = xi % S
    cls_q = bq * S + aq
    bank_q = ylq // 4
    # D row within a bank: xl*16 + t*4 + g (t = ylq%4, g = cls//4) so a
    # bank's D write is contiguous per partition xl
    grow = xlq * 16 + (ylq % 4) * 4 + cls_q // 4
    sub_q = cls_q % 4                                 # 3-float slot in row
    Q = coord.shape[0]

    host = dict(consts=None, im2cols=im2cols, Q=Q)
    plan = dict(
        dy_min=dy_min, dx_min=dx_min, KM=KM, KA=KA,
        NCF=NCF, NFK=NFK,
    )

    if GATHER:
        # Bank-pipelined sorted block-gather: per (core, bank), sort
        # queries by D row, split into 128 groups; partition p
        # block-fetches its span.
        per_cb = [[np.nonzero((core_of == core) & (bank_q == nb))[0]
                   for nb in range(4)] for core in range(NCORES)]
        NQBP = 128 * ((max(s.size for row in per_cb for s in row) + 127)
                      // 128)
        NQBP = max(NQBP, 128)
        idx_arrays, originals, subsels, localoff = [], [], [], []
        max_span = 1
        per = NQBP // 128
        for core in range(NCORES):
            lo4, org4, sub4, loc4 = [], [], [], []
            for nb in range(4):
                sel = per_cb[core][nb]
                rows = np.zeros(NQBP, np.int64)
                rows[:sel.size] = grow[sel]
                if sel.size:
                    rows[sel.size:] = rows[:sel.size].max()
                order = np.argsort(rows[:sel.size], kind='stable')
                rows_sorted = np.concatenate([rows[:sel.size][order],
                                              rows[sel.size:]])
                lo = rows_sorted.reshape(128, per)[:, 0].copy()
                span = rows_sorted.reshape(128, per)[:, -1] - lo + 1
                max_span = max(max_span, int(span.max()))
                lo4.append(lo)
                org4.append(sel[order])
                sub4.append(sub_q[sel][order])
                loc4.append(rows_sorted - np.repeat(lo, per))
            idx_arrays.append(lo4)
            originals.append(org4)
            subsels.append(sub4)
            localoff.append(loc4)
        BLK = min(NROWS_B, ((max_span + 3) // 4) * 4)
        for core in range(NCORES):
            lo4 = idx_arrays[core]
            for nb in range(4):
                lo = lo4[nb]
                lo2 = np.clip(np.minimum(lo, NROWS_B - BLK), 0, None)
                localoff[core][nb] = (
                    localoff[core][nb]
                    + np.repeat(lo - lo2, per)).astype(np.int64)
                assert (localoff[core][nb] < BLK).all()
                assert (localoff[core][nb] >= 0).all()
                lo4[nb] = lo2
            idx_arrays[core] = np.stack(
                [l.astype(np.int32) for l in lo4], axis=1)   # [128, 4]
        plan['BLK'] = BLK
        plan['NQBP'] = NQBP
        host.update(idx_arrays=idx_arrays, originals=originals,
                    subsels=subsels, localoff=localoff)
    else:
        host.update(core_of=core_of, cls_q=cls_q, ylq=ylq, xlq=xlq)

    # ---- pack constants into one [128, CW] blob ----
    segs = {}
    col = [0]

    def alloc(name, K, Mw):
        segs[name] = (0, col[0], K, Mw)
        col[0] += Mw

    for nb in range(4):
        alloc(f'Emb{nb}', KM, MROWS)
    alloc('corrT', MROWS, W)
    alloc('corrB', MROWS, W)
    CW = col[0]
    plan['segs'] = segs
    plan['CW'] = CW

    consts_cores = []
    for core in range(NCORES):
        blob = np.zeros((KM, CW), np.float32)
        for nb in range(4):
            tb = main_lhsT.copy()
            yls = YLC * core + 4 * nb + np.arange(4)
            tb[KA + 1:KA + 5, :] = corrL[:, yls].T
            tb[KA + 5:KM, :] = corrR[:, yls].T
            p0, c0, K, Mw = segs[f'Emb{nb}']
            blob[p0:p0 + K, c0:c0 + Mw] = tb
        p0, c0, K, Mw = segs['corrT']
        if core == 0:
            blob[p0:p0 + K, c0:c0 + Mw] = corrT
        p0, c0, K, Mw = segs['corrB']
        if core == NCORES - 1:
            blob[p0:p0 + K, c0:c0 + Mw] = corrB
        consts_cores.append(blob.astype(BF16))
    host['consts'] = consts_cores
    return plan, host


def _build_graph(plan, host, debug_outputs=False):
    _ensure_path()
    import concourse.bass as bass
    import concourse.bacc as bacc
    import concourse.mybir as mybir
    import concourse.tile as tile
    from concourse.masks import make_identity

    f32 = mybir.dt.float32
    bf16 = mybir.dt.bfloat16
    i32 = mybir.dt.int32

    KM, NCF, NFK = plan['KM'], plan['NCF'], plan['NFK']
    dx_min = plan['dx_min']
    segs, CW = plan['segs'], plan['CW']

    nc = bacc.Bacc(None, target_bir_lowering=False, debug=False,
                   num_devices=NCORES)

    imk_d = nc.dram_tensor('im2col', [KM, NFK], bf16, kind='ExternalInput')
    consts_d = nc.dram_tensor('consts', [KM, CW], bf16,
                              kind='ExternalInput')
    if GATHER:
        BLK = plan['BLK']
        idx_d = nc.dram_tensor('idx', [128, 4], i32, kind='ExternalInput')
        out_d = nc.dram_tensor('out', [128, 4 * BLK * 12], bf16,
                               kind='ExternalOutput')
    else:
        out_d = nc.dram_tensor('out', [MROWS, YLC * W], bf16,
                               kind='ExternalOutput')
    if debug_outputs:
        dbg_pred = nc.dram_tensor('dbg_pred', [MROWS, YLC * W], bf16,
                                  kind='ExternalOutput')

    with tile.TileContext(nc) as tc:
        with (
            tc.tile_pool(name='sb', bufs=1) as sb,
            tc.tile_pool(name='sbsmall', bufs=1) as sbs,
            tc.tile_pool(name='pshare', bufs=2, space='PSUM') as pshare,
            tc.tile_pool(name='ppred', bufs=1, space='PSUM') as ppred,
            tc.tile_pool(name='dram', bufs=1, space='DRAM') as dpool,
        ):
            consts_t = sb.tile([KM, CW], bf16)
            imk = sb.tile([KM, NFK], bf16)
            # consts first (small; first LDWEIGHTS needs it); imk in 4
            # row-group chunks (bank nb reads rows 4nb..4nb+4 only),
            # spread across engine queues so the ~0.8us descriptor
            # generation per DMA runs in parallel
            # scalar's queue is blocked ~1.3us by the auto ACT_TABLE_LOAD,
            # so everything the first matmuls need goes on sync
            if WARMUP_MM:
                # PE is otherwise idle for ~3us while the input DMAs land;
                # dummy matmuls (zeros, scratch PSUM) keep the HAM activity
                # window busy so the real mains run warm
                warm_sb = sbs.tile([1, 560], bf16)
                nc.vector.memset(warm_sb[:], 0.0)
                warm_ps = ppred.tile([MROWS, 512], f32, tag='warm',
                                     name='warm')
                for wi in range(WARMUP_MM):
                    nc.tensor.matmul(
                        warm_ps[:], warm_sb[:, 0:MROWS],
                        warm_sb[:, 48:560],
                        start=(wi == 0), stop=(wi == WARMUP_MM - 1),
                        skip_group_check=True)

            # consts on sync; imk chunk A on gpsimd so its descriptor
            # generation and transfer don't queue behind consts (gpsimd's
            # slower dispatch still beats serializing both on sync)
            # chunk arrival matches the PE bank order [0,3,1,2]:
            # A (bank0) on gpsimd, D (bank3) next on sync, then B, C
            RC = [0, 4 * NCF, 8 * NCF, 12 * NCF, NFK]
            nc.sync.dma_start(consts_t[:], consts_d[:])
            nc.gpsimd.dma_start(imk[:, RC[0]:RC[1]], imk_d[:, RC[0]:RC[1]])
            nc.sync.dma_start(imk[:, RC[3]:RC[4]], imk_d[:, RC[3]:RC[4]])
            nc.sync.dma_start(imk[:, RC[1]:RC[3]], imk_d[:, RC[1]:RC[3]])
            if GATHER:
                idx_t = sbs.tile([128, 4], i32)
                nc.scalar.dma_start(idx_t[:], idx_d[:])
                D_ts = [dpool.tile([NROWS_B, 12], bf16, tag=f'D{nb}',
                                   name=f'Dscr{nb}')
                        for nb in range(4)]
                D2s = [D_ts[nb][:].rearrange('(xl r) k -> xl (r k)', xl=128)
                       for nb in range(4)]
                gath = sb.tile([128, 4 * BLK * 12], bf16)

            def cseg(name):
                p0, c0, K, Mw = segs[name]
                return consts_t[p0:p0 + K, c0:c0 + Mw]

            ident = sbs.tile([MROWS, MROWS], bf16)
            make_identity(nc, ident[:])

            imk3 = imk[:].rearrange('p (r c) -> p r c', c=NCF)
            if GATHER:
                D_sb = sb.tile([128, YLC * MROWS], bf16)

            # one PSUM tile per bank so drains don't wait on later
            # banks' matmuls (Tile tracks deps at tile granularity)
            pred_ps = [ppred.tile([MROWS, 512], f32, tag=f'pp{nb}',
                                  name=f'pp{nb}')
                       for nb in range(4)]

            # ---- PE stream: one main per bank (+ top/bot rows for
            # banks 0/3); emit the edge banks first so their groups stop
            # early and the last-drained bank's (2) stop comes sooner ----
            for nb in [0, 3, 1, 2]:
                edge = EDGE_IN_MAIN and ((nb == 0) or (nb == 3))
                nc.tensor.matmul(
                    pred_ps[nb][:],
                    cseg(f'Emb{nb}'),
                    imk3[0:KM, 4 * nb:4 * nb + 4, 0:W],
                    start=True, stop=not edge,
                    skip_group_check=True)
                if edge and nb == 0:
                    nc.tensor.matmul(
                        pred_ps[0][:, 0:W], ident[:], cseg('corrT'),
                        start=False, stop=True, skip_group_check=True)
                if edge and nb == 3:
                    nc.tensor.matmul(
                        pred_ps[3][:, 3 * W:4 * W], ident[:],
                        cseg('corrB'),
                        start=False, stop=True, skip_group_check=True)

            # ---- per-bank drain -> transpose -> D copy -> D write ->
            # gather -> out ----
            pred_sb = sb.tile([MROWS, YLC * W], bf16)

            for oi, nb in enumerate([0, 3, 1, 2]):
                last = oi == 3
                # alternate banks between DVE and ACT so the two tail
                # chains run in parallel; the last bank (critical path)
                # splits across both
                if last and not NO_SCALAR:
                    nc.vector.tensor_copy(
                        pred_sb[:, nb * 512:nb * 512 + 256],
                        pred_ps[nb][:, 0:256])
                    nc.scalar.activation(
                        pred_sb[:, nb * 512 + 256:(nb + 1) * 512],
                        pred_ps[nb][:, 256:512],
                        mybir.ActivationFunctionType.Copy)
                elif NO_SCALAR or oi % 2 == 0:
                    nc.vector.tensor_copy(
                        pred_sb[:, nb * 512:(nb + 1) * 512],
                        pred_ps[nb][:])
                else:
                    nc.scalar.activation(
                        pred_sb[:, nb * 512:(nb + 1) * 512],
                        pred_ps[nb][:],
                        mybir.ActivationFunctionType.Copy)
                if not EDGE_IN_MAIN and nb == 0:
                    nc.vector.tensor_add(pred_sb[:, 0:W], pred_sb[:, 0:W],
                                         cseg('corrT'))
                if not EDGE_IN_MAIN and nb == 3:
                    nc.vector.tensor_add(pred_sb[:, (YLC - 1) * W:YLC * W],
                                         pred_sb[:, (YLC - 1) * W:YLC * W],
                                         cseg('corrB'))
                if GATHER:
                    pt = pshare.tile([128, 4 * MROWS], bf16, tag='pshare')
                    for t in range(4):
                        ch = 4 * nb + t
                        nc.tensor.transpose(
                            pt[:, t * MROWS:(t + 1) * MROWS],
                            pred_sb[:, ch * W:(ch + 1) * W], ident[:])
                    if NO_SCALAR or nb % 2 == 0:
                        nc.vector.tensor_copy(
                            D_sb[:, nb * 192:(nb + 1) * 192], pt[:, 0:192])
                    else:
                        nc.scalar.activation(
                            D_sb[:, nb * 192:(nb + 1) * 192], pt[:, 0:192],
                            mybir.ActivationFunctionType.Copy)
                    (nc.sync if nb % 2 == 0 else nc.scalar).dma_start(
                        D2s[nb][:, :], D_sb[:, nb * 192:(nb + 1) * 192])
                    nc.gpsimd.indirect_dma_start(
                        out=gath[:, nb * BLK * 12:(nb + 1) * BLK * 12],
                        out_offset=None,
                        in_=D_ts[nb][:],
                        in_offset=bass.IndirectOffsetOnAxis(
                            ap=idx_t[:, nb:nb + 1], axis=0))
                    nc.scalar.dma_start(
                        out_d[:, nb * BLK * 12:(nb + 1) * BLK * 12],
                        gath[:, nb * BLK * 12:(nb + 1) * BLK * 12])
                else:
                    # dump mode: no transpose needed -- the host picks
                    # straight from the class-major pred layout; last
                    # bank's write splits across both HWDGE queues
                    if last:
                        nc.sync.dma_start(
                            out_d[:, nb * 512:nb * 512 + 256],
                            pred_sb[:, nb * 512:nb * 512 + 256])
                        nc.scalar.dma_start(
                            out_d[:, nb * 512 + 256:(nb + 1) * 512],
                            pred_sb[:, nb * 512 + 256:(nb + 1) * 512])
                    else:
                        (nc.sync if oi % 2 == 0 else nc.scalar).dma_start(
                            out_d[:, nb * 512:(nb + 1) * 512],
                            pred_sb[:, nb * 512:(nb + 1) * 512])

            if debug_outputs:
                nc.sync.dma_start(dbg_pred[:], pred_sb[:])

    nc.compile()
    return nc


def make_in_maps(host):
    in_maps = []
    for core in range(NCORES):
        m = {
            'im2col': host['im2cols'][core],
            'consts': host['consts'][core],
        }
        if GATHER:
            m['idx'] = host['idx_arrays'][core]
        in_maps.append(m)
    return in_maps


def kernel(**inputs) -> np.ndarray:
    _ensure_path()
    from concourse.bass_utils import run_bass_kernel_spmd

    scale = inputs.get('scale', S)
    scale = int(np.asarray(scale)) if not isinstance(scale, int) else scale
    assert scale == S, f"kernel hardcodes scale={S}, got {scale}"

    plan, host = _plan_and_host_data(inputs)
    nc = _build_graph(plan, host)

    in_maps = make_in_maps(host)
    res = run_bass_kernel_spmd(nc, in_maps, core_ids=list(range(NCORES)))

    Q = host['Q']
    q = np.zeros((Q, 3), np.float32)
    if GATHER:
        BLK = plan['BLK']
        NQBP = plan['NQBP']
        per = NQBP // 128
        for core in range(NCORES):
            blocks = np.asarray(res.results[core]['out']).astype(
                np.float32).reshape(128, 4, BLK * 12)
            for nb in range(4):
                sel = host['originals'][core][nb]
                sub = host['subsels'][core][nb]
                loc = host['localoff'][core][nb]
                n = sel.size
                if n == 0:
                    continue
                prt = (np.arange(n) // per)
                base = loc[:n] * 12 + sub * 3
                cols = base[:, None] + np.arange(3)[None]
                q[sel] = np.take_along_axis(blocks[prt, nb], cols, axis=1)
    else:
        core_of = host['core_of']
        cls_q, ylq, xlq = host['cls_q'], host['ylq'], host['xlq']
        outs = np.stack([np.asarray(res.results[core]['out'])
                         for core in range(NCORES)]).astype(np.float32)
        # out[core][cls*3 + c, ylq*W + xlq]
        cols = ylq * W + xlq
        for c in range(3):
            q[:, c] = outs[core_of, cls_q * 3 + c, cols]
    return q[None]


# revision 59
# speedup vs baseline: 1.0541x; 1.0541x over previous
"""ArbSR (moe_routing) Trainium2 kernel, 8-core SPMD.

Structure exploited: with scale=4, the scale-embedding MLP input is periodic
with period 4 in both HR axes, so routing r, offsets off, and the expert-mix
matrices take only 16 distinct values (one per (y%4, x%4) class).  The
offset grid_sample is, per class, a 2x2-tap bilinear filter of the encoder
feature map at a constant integer shift, so the whole
  encoder conv -> fea0 -> expert mixing -> (+fea0) -> 3x3 tail conv
chain is LINEAR in the input image and collapses to a single 5x5-tap
convolution per (y%4, x%4) class:
  pred[:, 4*yl+b, 4*xl+a] = bias + sum_{ey,ex in 5x5} A[(ey,ex)] @
                            inp[:, yl+ey-2, xl+ex-2]
with host-precomputed [48, 3] matrices A (deltas composed with encoder
taps).  The device runs ONE K=84 matmul per bank of 4 LR rows (4 mains
total, N=512 each) against a 25-tap im2col of the raw input; the bias rides
an all-ones rhs K row.  The fold is exact at interior pixels; at image
borders (where the grid-sample zero-pad mask breaks linear folding) the
host computes exact (true - fold) corrections numerically: left/right
columns ride one-hot rhs K rows, top/bottom rows (cores 0/7) are
accumulated into PSUM by an identity-lhsT matmul against a [48, W] strip.

Per core (64 HR rows): 4 main + 2 edge matmuls, per-bank PSUM drains split
across DVE/ACT, and 4 direct class-major output writes; the host resolves
the nearest-neighbour queries from the dumped pred (query index math is
host-side, as in the original baseline).
"""

import numpy as np
import ml_dtypes

BF16 = ml_dtypes.bfloat16


def _ensure_path():
    import sys
    for p in ('/opt/trn_rl_repo',):
        if p not in sys.path:
            sys.path.append(p)


H = W = 128
S = 4
HH = WH = H * S          # 512
C = 64
NCORES = 8
YLC = H // NCORES        # 16 LR rows per core
HRPC = HH // NCORES      # 64 HR rows per core
NPIX = HRPC * WH         # 32768 HR pixels per core
NCLS = 16                # (b, a) classes
MROWS = NCLS * 3         # 48 stacked pred rows
KE = 28                  # encoder-folded contraction rows per dy block
NROWS_B = 2048           # D rows per bank (128 xl * 4 t * 4 g)

GATHER = False           # False: dump D as output, gather on host
EDGE_IN_MAIN = True      # fold top/bot row corr into the main PSUM group
T_SHARED = True          # transposes share one PSUM tile per bank
SPLIT_DRAIN = True       # split PSUM drains between Vector and Scalar
NO_SCALAR = False        # no scalar.activation at all: kills the 1.3us
                         # ACT_TABLE_LOAD that blocks scalar's DMA queue
WARMUP_MM = 3            # garbage matmuls while input DMAs land, to
                         # trigger the PE HAM un-throttle (1.2->2.4 GHz)
IMKA_SYNC = False        # imk chunk A on sync instead of gpsimd


def _sigmoid(x):
    return 1.0 / (1.0 + np.exp(-x))


def _class_constants(d):
    w1 = np.asarray(d['body_w1'], np.float64)
    b1 = np.asarray(d['body_b1'], np.float64)
    w2 = np.asarray(d['body_w2'], np.float64)
    b2 = np.asarray(d['body_b2'], np.float64)
    rw = np.asarray(d['routing_w'], np.float64)
    rb = np.asarray(d['routing_b'], np.float64)
    ow = np.asarray(d['offset_w'], np.float64)
    ob = np.asarray(d['offset_b'], np.float64)
    wc = np.asarray(d['weight_compress'], np.float64)
    we = np.asarray(d['weight_expand'], np.float64)

    fs = float(S)
    coor = np.array([(i + 0.5) / fs - np.floor((i + 0.5) / fs + 0.001) - 0.5
                     for i in range(S)])
    cls = {}
    for b in range(S):
        for a in range(S):
            inp4 = np.array([1.0 / fs, 1.0 / fs, coor[b], coor[a]])
            emb = np.maximum(w1 @ inp4 + b1, 0.0)
            emb = np.maximum(w2 @ emb + b2, 0.0)
            off = ow @ emb + ob
            r = _sigmoid(rw @ emb + rb)
            A = np.einsum('e,eck->ck', r, we) @ np.einsum('e,ekc->kc', r, wc)
            B = A + np.eye(C)
            cx = (a + 0.5) / fs - 0.5 + off[0]
            cy = (b + 0.5) / fs - 0.5 + off[1]
            ix, iy = int(np.floor(cx)), int(np.floor(cy))
            fx, fy = cx - ix, cy - iy
            wbl = {(0, 0): (1 - fy) * (1 - fx), (0, 1): (1 - fy) * fx,
                   (1, 0): fy * (1 - fx), (1, 1): fy * fx}
            cls[(b, a)] = dict(B=B, ix=ix, iy=iy, wbl=wbl)
    return cls


def _build_E(tail_w, cls):
    """E[(b,a)][(dy,dx)] = [3, C] so that pred contribution is E @ f(shift)."""
    Es = {}
    for b in range(S):
        for a in range(S):
            acc = {}
            for ty in range(3):
                for tx in range(3):
                    bp = (b + ty - 1) % S
                    oy = (b + ty - 1 - bp) // S
                    ap_ = (a + tx - 1) % S
                    ox = (a + tx - 1 - ap_) // S
                    c2 = cls[(bp, ap_)]
                    TB = tail_w[:, :, ty, tx] @ c2['B']
                    for (uy, ux), wgt in c2['wbl'].items():
                        if wgt == 0.0:
                            continue
                        key = (oy + c2['iy'] + uy, ox + c2['ix'] + ux)
                        acc[key] = acc.get(key, np.zeros((3, C))) + TB * wgt
            Es[(b, a)] = acc
    return Es


def _build_encw(d):
    """encw [28, C]: rows = 9 taps x 3 ch + inside-mask bias row."""
    enc_w = np.asarray(d['enc_w'], np.float64)
    enc_b = np.asarray(d['enc_b'], np.float64)
    encw = np.zeros((KE, C))
    for ty in range(3):
        for tx in range(3):
            for ch in range(3):
                encw[(ty * 3 + tx) * 3 + ch, :] = enc_w[:, ch, ty, tx]
    encw[27, :] = enc_b
    return encw


PADF = 4   # f64 f-map padding margin (covers all shift indexing)


def _host_f(d):
    """f64 encoder output, zero outside the image, with PADF margin."""
    inp = np.asarray(d['inp'], np.float64)[0]
    ip = np.pad(inp, ((0, 0), (1, 1), (1, 1)))
    enc_w = np.asarray(d['enc_w'], np.float64)
    enc_b = np.asarray(d['enc_b'], np.float64)
    f = np.zeros((C, H, W))
    for ty in range(3):
        for tx in range(3):
            f += np.einsum('oc,chw->ohw', enc_w[:, :, ty, tx],
                           ip[:, ty:ty + H, tx:tx + W])
    f += enc_b[:, None, None]
    fpad = np.zeros((C, H + 2 * PADF, W + 2 * PADF))
    fpad[:, PADF:PADF + H, PADF:PADF + W] = f
    return fpad


def _host_f_nomask(d):
    """f64 encoder output WITHOUT the zero-outside mask: enc taps on the
    zero-padded input + enc_b everywhere.  This is what the 5x5 fold
    computes; with PADF margin."""
    inp = np.asarray(d['inp'], np.float64)[0]
    P1 = PADF + 1
    ip = np.pad(inp, ((0, 0), (P1, P1), (P1, P1)))
    enc_w = np.asarray(d['enc_w'], np.float64)
    enc_b = np.asarray(d['enc_b'], np.float64)
    NE = H + 2 * PADF
    f = np.zeros((C, NE, NE))
    for ty in range(3):
        for tx in range(3):
            f += np.einsum('oc,chw->ohw', enc_w[:, :, ty, tx],
                           ip[:, ty:ty + NE, tx:tx + NE])
    f += enc_b[:, None, None]
    return f


def _zgrid(cls, fpad, yHs, xHs):
    """z = out2 + fea0 (zero outside the HR image) on a coordinate grid."""
    yh = np.asarray(yHs)
    xh = np.asarray(xHs)
    out = np.zeros((C, len(yh), len(xh)))
    for b_ in range(S):
        rm = np.nonzero(np.mod(yh, S) == b_)[0]
        if rm.size == 0:
            continue
        ys = yh[rm]
        yl = ys // S
        for a_ in range(S):
            cm = np.nonzero(np.mod(xh, S) == a_)[0]
            if cm.size == 0:
                continue
            xs = xh[cm]
            xl = xs // S
            inside = ((ys[:, None] >= 0) & (ys[:, None] < HH)
                      & (xs[None, :] >= 0) & (xs[None, :] < WH))
            c2 = cls[(b_, a_)]
            fg = np.zeros((C, rm.size, cm.size))
            for (uy, ux), wgt in c2['wbl'].items():
                rr = PADF + yl + c2['iy'] + uy
                cc = PADF + xl + c2['ix'] + ux
                fg += wgt * fpad[:, rr[:, None], cc[None, :]]
            val = np.einsum('oc,cyx->oyx', c2['B'], fg)
            val *= inside[None]
            out[np.ix_(np.arange(C), rm, cm)] = val
    return out


def _true_strip(cls, fpad, tail_w, tail_b, ylgs, xs_lr):
    """Exact pred values [MROWS, len(ylgs), len(xs_lr)] (LR coords)."""
    ylgs = np.asarray(ylgs)
    xs_lr = np.asarray(xs_lr)
    yHs = np.arange(S * ylgs.min() - 1, S * ylgs.max() + S + 1)
    xHs = np.arange(S * xs_lr.min() - 1, S * xs_lr.max() + S + 1)
    z = _zgrid(cls, fpad, yHs, xHs)
    y0, x0 = yHs[0], xHs[0]
    out = np.zeros((MROWS, len(ylgs), len(xs_lr)))
    for b in range(S):
        for a in range(S):
            m0 = (4 * b + a) * 3
            acc = np.zeros((3, len(ylgs), len(xs_lr)))
            for ty in range(3):
                rr = S * ylgs + b + ty - 1 - y0
                for tx in range(3):
                    cc = S * xs_lr + a + tx - 1 - x0
                    acc += np.einsum(
                        'oc,cyx->oyx', tail_w[:, :, ty, tx],
                        z[:, rr[:, None], cc[None, :]])
            out[m0:m0 + 3] = acc + tail_b[:, None, None]
    return out


def _fold_strip(G_main, bias48, fpad, ylgs, xs_lr):
    """What the device mains+bias compute, in f64 (LR coords)."""
    ylgs = np.asarray(ylgs)
    xs_lr = np.asarray(xs_lr)
    out = np.zeros((MROWS, len(ylgs), len(xs_lr)))
    for (dy, dx), G in G_main.items():
        rr = PADF + ylgs + dy
        cc = PADF + xs_lr + dx
        out += np.einsum('mc,cyx->myx', G, fpad[:, rr[:, None], cc[None, :]])
    return out + bias48[:, None, None]


def _plan_and_host_data(d):
    """Host precompute: folded lhsT matrices, per-core im2colK, edge
    corrections, query routing."""
    cls = _class_constants(d)
    tail_w = np.asarray(d['tail_w'], np.float64)
    tail_b = np.asarray(d['tail_b'], np.float64)
    encw = _build_encw(d)

    E_main = _build_E(tail_w, cls)
    deltas = sorted({k for acc in E_main.values() for k in acc})
    dys = sorted({dl[0] for dl in deltas})
    dxs = sorted({dl[1] for dl in deltas})
    dy_min, dy_max = min(dys), max(dys)
    dx_min, dx_max = min(dxs), max(dxs)
    assert dy_min >= -1 and dy_max <= 1 and dx_min >= -1 and dx_max <= 1
    # full 5x5 fold: delta (3x3) composed with the encoder taps (3x3)
    # gives pred = sum_{ey,ex in 5x5} A[ey,ex] @ inp + bias, valid at
    # interior pixels (the f zero-pad mask is 1); borders are fixed by
    # the numeric true-fold corrections
    KA = 75                          # 25 input taps x 3 channels
    KM = KA + 1 + 8                  # + bias row + lef/rig one-hots
    NCF = W                          # no column margins: taps carry them
    NFK = YLC * NCF

    G_main = {dl: np.zeros((MROWS, C)) for dl in deltas}
    for (b, a), acc in E_main.items():
        m0 = (4 * b + a) * 3
        for dl, M in acc.items():
            G_main[dl][m0:m0 + 3, :] += M

    bias48 = np.zeros(MROWS)
    for b in range(S):
        for a in range(S):
            bias48[(4 * b + a) * 3:(4 * b + a) * 3 + 3] = tail_b

    # ---- exact edge corrections (true - fold), f64 on host ----
    fpad = _host_f(d)
    fnom = _host_f_nomask(d)
    all_yl = np.arange(H)
    t_lef = _true_strip(cls, fpad, tail_w, tail_b, all_yl, [0])[:, :, 0]
    t_rig = _true_strip(cls, fpad, tail_w, tail_b, all_yl, [W - 1])[:, :, 0]
    f_lef = _fold_strip(G_main, bias48, fnom, all_yl, [0])[:, :, 0]
    f_rig = _fold_strip(G_main, bias48, fnom, all_yl, [W - 1])[:, :, 0]
    corrL = t_lef - f_lef            # [MROWS, H]
    corrR = t_rig - f_rig
    all_x = np.arange(W)
    t_top = _true_strip(cls, fpad, tail_w, tail_b, [0], all_x)[:, 0, :]
    f_top = _fold_strip(G_main, bias48, fnom, [0], all_x)[:, 0, :]
    corrT = t_top - f_top            # [MROWS, W]
    corrT[:, 0] -= corrL[:, 0]
    corrT[:, W - 1] -= corrR[:, 0]
    t_bot = _true_strip(cls, fpad, tail_w, tail_b, [H - 1], all_x)[:, 0, :]
    f_bot = _fold_strip(G_main, bias48, fnom, [H - 1], all_x)[:, 0, :]
    corrB = t_bot - f_bot
    corrB[:, 0] -= corrL[:, H - 1]
    corrB[:, W - 1] -= corrR[:, H - 1]

    # ---- main lhsT: one [KM, MROWS] via 5x5 A-matrices ----
    # row (ey*5+ex)*3+c accumulates F[(ty*3+tx)*3+c] over all
    # (dy,dx),(ty,tx) with ey=dy+ty-1+2, ex=dx+tx-1+2; F row 27
    # (enc-bias term, mask==1 interior) folds into the bias row
    main_lhsT = np.zeros((KM, MROWS))
    bias_row = bias48.copy()
    for (dy, dx), G in G_main.items():
        F = encw @ G.T
        for ty in range(3):
            for tx in range(3):
                ey, ex = dy + ty - 1 + 2, dx + tx - 1 + 2
                for c in range(3):
                    main_lhsT[(ey * 5 + ex) * 3 + c, :] += \
                        F[(ty * 3 + tx) * 3 + c, :]
        bias_row += F[27, :]
    main_lhsT[KA, :] = bias_row

    # ---- per-core im2col5 [KM, YLC*W]: 25 raw-input taps ----
    inp = np.asarray(d['inp'], np.float64)[0]   # [3, H, W]
    PADX = 8
    ippad = np.pad(inp, ((0, 0), (PADX, PADX), (PADX, PADX)))
    im2cols = []
    for core in range(NCORES):
        y0 = YLC * core
        imk = np.zeros((KM, YLC, NCF), np.float32)
        for ey in range(5):
            for ex in range(5):
                ys = PADX + y0 + ey - 2
                xs = PADX + ex - 2
                for ch in range(3):
                    imk[(ey * 5 + ex) * 3 + ch] = \
                        ippad[ch, ys:ys + YLC, xs:xs + NCF]
        imk[KA] = 1.0                              # bias row
        for yl in range(YLC):                      # lef/rig one-hot rows
            imk[KA + 1 + yl % 4, yl, 0] = 1.0
            imk[KA + 5 + yl % 4, yl, W - 1] = 1.0
        im2cols.append(imk.reshape(KM, NFK).astype(BF16))

    # ---- query routing (f32 math matches reference rounding) ----
    coord = np.asarray(d['coord'], np.float32)[0]
    cell = np.asarray(d['cell'], np.float32)[0]
    cq = np.clip(coord - cell * np.float32(0.5) + np.float32(1e-6),
                 np.float32(-1 + 1e-6), np.float32(1 - 1e-6))
    xi = np.clip(np.round((cq[:, 1] + 1) * np.float32(0.5) * (WH - 1)
                          ).astype(np.int64), 0, WH - 1)
    yi = np.clip(np.round((cq[:, 0] + 1) * np.float32(0.5) * (HH - 1)
                          ).astype(np.int64), 0, HH - 1)
    core_of = yi // HRPC
    ylq = (yi % HRPC) // S
    bq = yi % S
    xlq = xi // S
    aq = xi % S
    cls_q = bq * S + aq
    bank_q = ylq // 4
    # D row within a bank: xl*16 + t*4 + g (t = ylq%4, g = cls//4) so a
    # bank's D write is contiguous per partition xl
    grow = xlq * 16 + (ylq % 4) * 4 + cls_q // 4
    sub_q = cls_q % 4                                 # 3-float slot in row
    Q = coord.shape[0]

    host = dict(consts=None, im2cols=im2cols, Q=Q)
    plan = dict(
        dy_min=dy_min, dx_min=dx_min, KM=KM, KA=KA,
        NCF=NCF, NFK=NFK,
    )

    if GATHER:
        # Bank-pipelined sorted block-gather: per (core, bank), sort
        # queries by D row, split into 128 groups; partition p
        # block-fetches its span.
        per_cb = [[np.nonzero((core_of == core) & (bank_q == nb))[0]
                   for nb in range(4)] for core in range(NCORES)]
        NQBP = 128 * ((max(s.size for row in per_cb for s in row) + 127)
                      // 128)
        NQBP = max(NQBP, 128)
        idx_arrays, originals, subsels, localoff = [], [], [], []
        max_span = 1
        per = NQBP // 128
        for core in range(NCORES):
            lo4, org4, sub4, loc4 = [], [], [], []
            for nb in range(4):
                sel = per_cb[core][nb]
                rows = np.zeros(NQBP, np.int64)
                rows[:sel.size] = grow[sel]
                if sel.size:
                    rows[sel.size:] = rows[:sel.size].max()
                order = np.argsort(rows[:sel.size], kind='stable')
                rows_sorted = np.concatenate([rows[:sel.size][order],
                                              rows[sel.size:]])
                lo = rows_sorted.reshape(128, per)[:, 0].copy()
                span = rows_sorted.reshape(128, per)[:, -1] - lo + 1
                max_span = max(max_span, int(span.max()))
                lo4.append(lo)
                org4.append(sel[order])
                sub4.append(sub_q[sel][order])
                loc4.append(rows_sorted - np.repeat(lo, per))
            idx_arrays.append(lo4)
            originals.append(org4)
            subsels.append(sub4)
            localoff.append(loc4)
        BLK = min(NROWS_B, ((max_span + 3) // 4) * 4)
        for core in range(NCORES):
            lo4 = idx_arrays[core]
            for nb in range(4):
                lo = lo4[nb]
                lo2 = np.clip(np.minimum(lo, NROWS_B - BLK), 0, None)
                localoff[core][nb] = (
                    localoff[core][nb]
                    + np.repeat(lo - lo2, per)).astype(np.int64)
                assert (localoff[core][nb] < BLK).all()
                assert (localoff[core][nb] >= 0).all()
                lo4[nb] = lo2
            idx_arrays[core] = np.stack(
                [l.astype(np.int32) for l in lo4], axis=1)   # [128, 4]
        plan['BLK'] = BLK
        plan['NQBP'] = NQBP
        host.update(idx_arrays=idx_arrays, originals=originals,
                    subsels=subsels, localoff=localoff)
    else:
        host.update(core_of=core_of, cls_q=cls_q, ylq=ylq, xlq=xlq)

    # ---- pack constants into one [128, CW] blob ----
    segs = {}
    col = [0]

    def alloc(name, K, Mw):
        segs[name] = (0, col[0], K, Mw)
        col[0] += Mw

    for nb in range(4):
        alloc(f'Emb{nb}', KM, MROWS)
    alloc('corrT', MROWS, W)
    alloc('corrB', MROWS, W)
    CW = col[0]
    plan['segs'] = segs
    plan['CW'] = CW

    consts_cores = []
    for core in range(NCORES):
        blob = np.zeros((KM, CW), np.float32)
        for nb in range(4):
            tb = main_lhsT.copy()
            yls = YLC * core + 4 * nb + np.arange(4)
            tb[KA + 1:KA + 5, :] = corrL[:, yls].T
            tb[KA + 5:KM, :] = corrR[:, yls].T
            p0, c0, K, Mw = segs[f'Emb{nb}']
            blob[p0:p0 + K, c0:c0 + Mw] = tb
        p0, c0, K, Mw = segs['corrT']
        if core == 0:
            blob[p0:p0 + K, c0:c0 + Mw] = corrT
        p0, c0, K, Mw = segs['corrB']
        if core == NCORES - 1:
            blob[p0:p0 + K, c0:c0 + Mw] = corrB
        consts_cores.append(blob.astype(BF16))
    host['consts'] = consts_cores
    return plan, host


def _build_graph(plan, host, debug_outputs=False):
    _ensure_path()
    import concourse.bass as bass
    import concourse.bacc as bacc
    import concourse.mybir as mybir
    import concourse.tile as tile
    from concourse.masks import make_identity

    f32 = mybir.dt.float32
    bf16 = mybir.dt.bfloat16
    i32 = mybir.dt.int32

    KM, NCF, NFK = plan['KM'], plan['NCF'], plan['NFK']
    dx_min = plan['dx_min']
    segs, CW = plan['segs'], plan['CW']

    nc = bacc.Bacc(None, target_bir_lowering=False, debug=False,
                   num_devices=NCORES)

    imk_d = nc.dram_tensor('im2col', [KM, NFK], bf16, kind='ExternalInput')
    consts_d = nc.dram_tensor('consts', [KM, CW], bf16,
                              kind='ExternalInput')
    if GATHER:
        BLK = plan['BLK']
        idx_d = nc.dram_tensor('idx', [128, 4], i32, kind='ExternalInput')
        out_d = nc.dram_tensor('out', [128, 4 * BLK * 12], bf16,
                               kind='ExternalOutput')
    else:
        out_d = nc.dram_tensor('out', [MROWS, YLC * W], bf16,
                               kind='ExternalOutput')
    if debug_outputs:
        dbg_pred = nc.dram_tensor('dbg_pred', [MROWS, YLC * W], bf16,
                                  kind='ExternalOutput')

    with tile.TileContext(nc) as tc:
        with (
            tc.tile_pool(name='sb', bufs=1) as sb,
            tc.tile_pool(name='sbsmall', bufs=1) as sbs,
            tc.tile_pool(name='pshare', bufs=2, space='PSUM') as pshare,
            tc.tile_pool(name='ppred', bufs=1, space='PSUM') as ppred,
            tc.tile_pool(name='dram', bufs=1, space='DRAM') as dpool,
        ):
            consts_t = sb.tile([KM, CW], bf16)
            imk = sb.tile([KM, NFK], bf16)
            # consts first (small; first LDWEIGHTS needs it); imk in 4
            # row-group chunks (bank nb reads rows 4nb..4nb+4 only),
            # spread across engine queues so the ~0.8us descriptor
            # generation per DMA runs in parallel
            # scalar's queue is blocked ~1.3us by the auto ACT_TABLE_LOAD,
            # so everything the first matmuls need goes on sync
            if WARMUP_MM:
                # PE is otherwise idle for ~3us while the input DMAs land;
                # dummy matmuls (zeros, scratch PSUM) keep the HAM activity
                # window busy so the real mains run warm
                warm_sb = sbs.tile([1, 560], bf16)
                nc.vector.memset(warm_sb[:], 0.0)
                warm_ps = ppred.tile([MROWS, 512], f32, tag='warm',
                                     name='warm')
                for wi in range(WARMUP_MM):
                    nc.tensor.matmul(
                        warm_ps[:], warm_sb[:, 0:MROWS],
                        warm_sb[:, 48:560],
                        start=(wi == 0), stop=(wi == WARMUP_MM - 1),
                        skip_group_check=True)

            # consts on sync; imk chunk A on gpsimd so its descriptor
            # generation and transfer don't queue behind consts (gpsimd's
            # slower dispatch still beats serializing both on sync)
            # chunk arrival matches the PE bank order [0,3,1,2]:
            # A (bank0) on gpsimd, D (bank3) next on sync, then B, C
            RC = [0, 4 * NCF, 8 * NCF, 12 * NCF, NFK]
            nc.sync.dma_start(consts_t[:], consts_d[:])
            nc.gpsimd.dma_start(imk[:, RC[0]:RC[1]], imk_d[:, RC[0]:RC[1]])
            nc.sync.dma_start(imk[:, RC[3]:RC[4]], imk_d[:, RC[3]:RC[4]])
            nc.sync.dma_start(imk[:, RC[1]:RC[2]], imk_d[:, RC[1]:RC[2]])
            nc.scalar.dma_start(imk[:, RC[2]:RC[3]], imk_d[:, RC[2]:RC[3]])
            if GATHER:
                idx_t = sbs.tile([128, 4], i32)
                nc.scalar.dma_start(idx_t[:], idx_d[:])
                D_ts = [dpool.tile([NROWS_B, 12], bf16, tag=f'D{nb}',
                                   name=f'Dscr{nb}')
                        for nb in range(4)]
                D2s = [D_ts[nb][:].rearrange('(xl r) k -> xl (r k)', xl=128)
                       for nb in range(4)]
                gath = sb.tile([128, 4 * BLK * 12], bf16)

            def cseg(name):
                p0, c0, K, Mw = segs[name]
                return consts_t[p0:p0 + K, c0:c0 + Mw]

            ident = sbs.tile([MROWS, MROWS], bf16)
            make_identity(nc, ident[:])

            imk3 = imk[:].rearrange('p (r c) -> p r c', c=NCF)
            if GATHER:
                D_sb = sb.tile([128, YLC * MROWS], bf16)

            # one PSUM tile per bank so drains don't wait on later
            # banks' matmuls (Tile tracks deps at tile granularity)
            pred_ps = [ppred.tile([MROWS, 512], f32, tag=f'pp{nb}',
                                  name=f'pp{nb}')
                       for nb in range(4)]

            # ---- PE stream: one main per bank (+ top/bot rows for
            # banks 0/3); emit the edge banks first so their groups stop
            # early and the last-drained bank's (2) stop comes sooner ----
            for nb in [0, 3, 1, 2]:
                edge = EDGE_IN_MAIN and ((nb == 0) or (nb == 3))
                nc.tensor.matmul(
                    pred_ps[nb][:],
                    cseg(f'Emb{nb}'),
                    imk3[0:KM, 4 * nb:4 * nb + 4, 0:W],
                    start=True, stop=not edge,
                    skip_group_check=True)
                if edge and nb == 0:
                    nc.tensor.matmul(
                        pred_ps[0][:, 0:W], ident[:], cseg('corrT'),
                        start=False, stop=True, skip_group_check=True)
                if edge and nb == 3:
                    nc.tensor.matmul(
                        pred_ps[3][:, 3 * W:4 * W], ident[:],
                        cseg('corrB'),
                        start=False, stop=True, skip_group_check=True)

            # ---- per-bank drain -> transpose -> D copy -> D write ->
            # gather -> out ----
            pred_sb = sb.tile([MROWS, YLC * W], bf16)

            for oi, nb in enumerate([0, 3, 1, 2]):
                last = oi == 3
                # alternate banks between DVE and ACT so the two tail
                # chains run in parallel; the last bank (critical path)
                # splits across both
                if last and not NO_SCALAR:
                    nc.vector.tensor_copy(
                        pred_sb[:, nb * 512:nb * 512 + 256],
                        pred_ps[nb][:, 0:256])
                    nc.scalar.activation(
                        pred_sb[:, nb * 512 + 256:(nb + 1) * 512],
                        pred_ps[nb][:, 256:512],
                        mybir.ActivationFunctionType.Copy)
                elif NO_SCALAR or oi % 2 == 0:
                    nc.vector.tensor_copy(
                        pred_sb[:, nb * 512:(nb + 1) * 512],
                        pred_ps[nb][:])
                else:
                    nc.scalar.activation(
                        pred_sb[:, nb * 512:(nb + 1) * 512],
                        pred_ps[nb][:],
                        mybir.ActivationFunctionType.Copy)
                if not EDGE_IN_MAIN and nb == 0:
                    nc.vector.tensor_add(pred_sb[:, 0:W], pred_sb[:, 0:W],
                                         cseg('corrT'))
                if not EDGE_IN_MAIN and nb == 3:
                    nc.vector.tensor_add(pred_sb[:, (YLC - 1) * W:YLC * W],
                                         pred_sb[:, (YLC - 1) * W:YLC * W],
                                         cseg('corrB'))
                if GATHER:
                    pt = pshare.tile([128, 4 * MROWS], bf16, tag='pshare')
                    for t in range(4):
                        ch = 4 * nb + t
                        nc.tensor.transpose(
                            pt[:, t * MROWS:(t + 1) * MROWS],
                            pred_sb[:, ch * W:(ch + 1) * W], ident[:])
                    if NO_SCALAR or nb % 2 == 0:
                        nc.vector.tensor_copy(
                            D_sb[:, nb * 192:(nb + 1) * 192], pt[:, 0:192])
                    else:
                        nc.scalar.activation(
                            D_sb[:, nb * 192:(nb + 1) * 192], pt[:, 0:192],
                            mybir.ActivationFunctionType.Copy)
                    (nc.sync if nb % 2 == 0 else nc.scalar).dma_start(
                        D2s[nb][:, :], D_sb[:, nb * 192:(nb + 1) * 192])
                    nc.gpsimd.indirect_dma_start(
                        out=gath[:, nb * BLK * 12:(nb + 1) * BLK * 12],
                        out_offset=None,
                        in_=D_ts[nb][:],
                        in_offset=bass.IndirectOffsetOnAxis(
                            ap=idx_t[:, nb:nb + 1], axis=0))
                    nc.scalar.dma_start(
                        out_d[:, nb * BLK * 12:(nb + 1) * BLK * 12],
                        gath[:, nb * BLK * 12:(nb + 1) * BLK * 12])
                else:
                    # dump mode: no transpose needed -- the host picks
                    # straight from the class-major pred layout; last
                    # bank's write splits across both HWDGE queues
                    if last:
                        nc.sync.dma_start(
                            out_d[:, nb * 512:nb * 512 + 256],
                            pred_sb[:, nb * 512:nb * 512 + 256])
                        nc.scalar.dma_start(
                            out_d[:, nb * 512 + 256:(nb + 1) * 512],
                            pred_sb[:, nb * 512 + 256:(nb + 1) * 512])
                    else:
                        (nc.sync if oi % 2 == 0 else nc.scalar).dma_start(
                            out_d[:, nb * 512:(nb + 1) * 512],
                            pred_sb[:, nb * 512:(nb + 1) * 512])

            if debug_outputs:
                nc.sync.dma_start(dbg_pred[:], pred_sb[:])

    nc.compile()
    return nc


def make_in_maps(host):
    in_maps = []
    for core in range(NCORES):
        m = {
            'im2col': host['im2cols'][core],
            'consts': host['consts'][core],
        }
        if GATHER:
            m['idx'] = host['idx_arrays'][core]
        in_maps.append(m)
    return in_maps


def kernel(**inputs) -> np.ndarray:
    _ensure_path()
    from concourse.bass_utils import run_bass_kernel_spmd

    scale = inputs.get('scale', S)
    scale = int(np.asarray(scale)) if not isinstance(scale, int) else scale
    assert scale == S, f"kernel hardcodes scale={S}, got {scale}"

    plan, host = _plan_and_host_data(inputs)
    nc = _build_graph(plan, host)

    in_maps = make_in_maps(host)
    res = run_bass_kernel_spmd(nc, in_maps, core_ids=list(range(NCORES)))

    Q = host['Q']
    q = np.zeros((Q, 3), np.float32)
    if GATHER:
        BLK = plan['BLK']
        NQBP = plan['NQBP']
        per = NQBP // 128
        for core in range(NCORES):
            blocks = np.asarray(res.results[core]['out']).astype(
                np.float32).reshape(128, 4, BLK * 12)
            for nb in range(4):
                sel = host['originals'][core][nb]
                sub = host['subsels'][core][nb]
                loc = host['localoff'][core][nb]
                n = sel.size
                if n == 0:
                    continue
                prt = (np.arange(n) // per)
                base = loc[:n] * 12 + sub * 3
                cols = base[:, None] + np.arange(3)[None]
                q[sel] = np.take_along_axis(blocks[prt, nb], cols, axis=1)
    else:
        core_of = host['core_of']
        cls_q, ylq, xlq = host['cls_q'], host['ylq'], host['xlq']
        outs = np.stack([np.asarray(res.results[core]['out'])
                         for core in range(NCORES)]).astype(np.float32)
        # out[core][cls*3 + c, ylq*W + xlq]
        cols = ylq * W + xlq
        for c in range(3):
            q[:, c] = outs[core_of, cls_q * 3 + c, cols]
    return q[None]
